# revision 41
# baseline (speedup 1.0000x reference)
"""Bass/Trainium2 kernel for nn_ExpMovAvgModel (sparse_attention).

Math (per batch row b, query t, key s, H=128 hidden):
    x      = embd[seq]                        # [T, H] gathered rows
    xhat   = x / |x|                          # row-normalized
    raw    = xhat @ xhat.T                    # cosine similarity [T, T]
    sim01  = 0.5*(raw+1) masked to s < t
    delta  = reversed-cumsum_s(sim01)         # = sum_{v=s}^{t-1} sim01[v]
    lam    = exp(x @ lam_w + lam_b)
    w      = sim01 * exp(-lam*delta)
    yhat   = clip((w @ y) / (sum_s w + 1e-6), 0.01, 0.99)

Key restructure: with q[u] = exp(-lam*sim01[u]), the forward scan
    S[s] = (S[s-1] + (raw[u]+1)) * q[s]
satisfies S[t-1] = 2*sum_s w[t,s]; with data0 = (raw[u]+1)*y[u] it gives
2*(w @ y).  One tensor_tensor_scan per row-block replaces the masked
reversed cumsum, the [T,T] exp, the weight multiply and the row
reduction.  The strict-causal mask is applied only to the 128-wide
diagonal block, and masked positions get data0=0, q=exp(0)=1 so the
scan state FREEZES at s=t-1: the last scan column is the answer for
every row (no per-row diagonal extraction).

Engine placement (measured): scans + diag masks on DVE; exp on ACT; all
(raw+1) / (raw+1)*y tiles are built by the PE itself - the sim matmul,
a second matmul against y-scaled embeddings, and K=1 all-ones matmuls
that add +1 / +y[s] into PSUM - so the scans read PSUM directly and DVE
does almost nothing but scan.  GPSIMD only runs the embedding gather
(its compute throughput is terrible).  Matmuls run in float32r
(1 cycle/row at N>=256 vs 4 for fp32; ~2e-3 rel err, well in budget).

Sharding: data-parallel over batch B=32 -> 4 batches per core x 8 cores.
"""

import os
import sys

import numpy as np

for _p in ("/opt/trn_rl_repo",):
    if _p not in sys.path and os.path.isdir(_p):
        sys.path.append(_p)

import concourse.bass as bass
import concourse.tile as tile
from concourse import bacc, mybir
from concourse.bass_utils import run_bass_kernel_spmd

P = 128            # partitions / hidden dim
T = 1024           # sequence length
NJ = T // P        # 8 column-blocks
NB_PER_CORE = 4    # batches per core
N_CORES = 8
N_VOCAB = 50000

F32 = mybir.dt.float32
I32 = mybir.dt.int32
MM_DTYPE = mybir.dt.float32r


def build_program():
    nc = bacc.Bacc(
        "TRN2",
        target_bir_lowering=False,
        debug=False,
        num_devices=N_CORES,
    )

    table = nc.dram_tensor("table", [N_VOCAB, P], F32, kind="ExternalInput").ap()
    idx = nc.dram_tensor("idx", [NB_PER_CORE, P, NJ], I32, kind="ExternalInput").ap()
    ybc = nc.dram_tensor("ybc", [NB_PER_CORE, P, T], F32, kind="ExternalInput").ap()
    ycol = nc.dram_tensor("ycol", [NB_PER_CORE, P, NJ], F32, kind="ExternalInput").ap()
    lamw = nc.dram_tensor("lamw", [P, 1], F32, kind="ExternalInput").ap()
    lamb = nc.dram_tensor("lamb", [P, 1], F32, kind="ExternalInput").ap()
    diag = nc.dram_tensor("diag", [P, P], F32, kind="ExternalInput").ap()
    smask = nc.dram_tensor("smask", [P, P], F32, kind="ExternalInput").ap()
    minfw = nc.dram_tensor("minfw", [P, P], F32, kind="ExternalInput").ap()
    minfy = nc.dram_tensor("minfy", [P, P], F32, kind="ExternalInput").ap()
    out = nc.dram_tensor("out", [NB_PER_CORE, P, NJ], F32, kind="ExternalOutput").ap()

    with tile.TileContext(nc) as tc:
        _build_body(tc, table, idx, ybc, ycol, lamw, lamb, diag, smask, minfw, minfy, out)

    nc.compile()
    return nc


def _build_body(tc, table, idx, ybc, ycol, lamw, lamb, diag, smask, minfw, minfy, out):
    from contextlib import ExitStack

    nc = tc.nc
    Exp = mybir.ActivationFunctionType.Exp
    Sqrt = mybir.ActivationFunctionType.Sqrt
    ADD = mybir.AluOpType.add
    MULT = mybir.AluOpType.mult
    MAX = mybir.AluOpType.max
    MIN = mybir.AluOpType.min

    with ExitStack() as ctx:
        pconst = ctx.enter_context(tc.tile_pool(name="pconst", bufs=1))
        pbatch = ctx.enter_context(tc.tile_pool(name="pbatch", bufs=2))
        psmall = ctx.enter_context(tc.tile_pool(name="psmall", bufs=2))
        pmain = ctx.enter_context(tc.tile_pool(name="pmain", bufs=3))
        pscan = ctx.enter_context(tc.tile_pool(name="pscan", bufs=5))
        pps = ctx.enter_context(tc.tile_pool(name="pps", bufs=3, space="PSUM"))
        ppsy = ctx.enter_context(tc.tile_pool(name="ppsy", bufs=2, space="PSUM"))
        ppsx = ctx.enter_context(tc.tile_pool(name="ppsx", bufs=2, space="PSUM"))
        ppsl = ctx.enter_context(tc.tile_pool(name="ppsl", bufs=1, space="PSUM"))

        diag_sb = pconst.tile([P, P], F32)
        nc.sync.dma_start(out=diag_sb[:], in_=diag)
        lamw_sb = pconst.tile([P, 1], F32)
        nc.sync.dma_start(out=lamw_sb[:], in_=lamw)
        lamb_sb = pconst.tile([P, 1], F32)
        nc.sync.dma_start(out=lamb_sb[:], in_=lamb)
        smask_sb = pconst.tile([P, P], F32)
        nc.sync.dma_start(out=smask_sb[:], in_=smask)
        # min-masks: clamping the diag PSUM block with these makes the masked
        # positions produce exactly the frozen-scan values through the regular
        # ACT ops: raw=-1 -> q=exp(0)=1, rawp1=0; rawy=0 (valid since y>0)
        minfw_sb = pconst.tile([P, P], F32)
        nc.sync.dma_start(out=minfw_sb[:], in_=minfw)
        minfy_sb = pconst.tile([P, P], F32)
        nc.sync.dma_start(out=minfy_sb[:], in_=minfy)
        # all-(1/128) matrix: K=128 matmul against a broadcast y tile adds
        # +y[s] to every row of a PSUM at full PE rate (K=1 matmuls are slow)
        inv_f32 = pconst.tile([P, P], F32)
        nc.vector.memset(inv_f32[:], 1.0 / P)
        inv_mm = pconst.tile([P, P], MM_DTYPE)
        nc.scalar.copy(out=inv_mm[:], in_=inv_f32[:])
        onesr_f32 = pconst.tile([P, 512], F32)
        nc.vector.memset(onesr_f32[:], 1.0)
        onesr = pconst.tile([P, 512], MM_DTYPE)
        nc.scalar.copy(out=onesr[:], in_=onesr_f32[:])

        for b in range(NB_PER_CORE):
            # ---- gather x rows: xg_j[p, :] = table[seq[j*128+p]] ----
            # (HW indirect DMA: one index per partition, contiguous out)
            idx_sb = psmall.tile([P, NJ], I32, tag="idx_sb")
            nc.sync.dma_start(out=idx_sb[:], in_=idx[b])
            xgs = []
            for j in range(NJ):
                xg = pbatch.tile([P, P], F32, tag=f"xg{j}")
                nc.gpsimd.indirect_dma_start(
                    out=xg[:],
                    out_offset=None,
                    in_=table,
                    in_offset=bass.IndirectOffsetOnAxis(
                        ap=idx_sb[:, j : j + 1], axis=0
                    ),
                )
                xgs.append(xg)

            # ---- norms, normalize, transpose, lam ----
            # batch 0 runs a per-j pipeline (tb=0 can start right after the
            # first gather -> kills the cold-start stall); later batches use
            # batched ops (fewer ACT table switches / small-op overheads)
            magsq = psmall.tile([P, NJ], F32, tag="magsq")
            mag = psmall.tile([P, NJ], F32, tag="mag")
            rmag = psmall.tile([P, NJ], F32, tag="rmag")
            xhat = pbatch.tile([P, NJ, P], F32, tag="xhat")
            xhaty = pbatch.tile([P, NJ, P], F32, tag="xhaty")
            xhatT = pbatch.tile([P, T], MM_DTYPE, tag="xhatT")
            xhatTY = pbatch.tile([P, T], MM_DTYPE, tag="xhatTY")
            ycol_sb = psmall.tile([P, NJ], F32, tag="ycol_sb")
            nc.sync.dma_start(out=ycol_sb[:], in_=ycol[b])
            rmy = psmall.tile([P, NJ], F32, tag="rmy")
            lamdot_ps = ppsl.tile([P, NJ], F32, tag="lamdot_ps")
            dotm = psmall.tile([P, NJ], F32, tag="dotm")
            lam = psmall.tile([P, NJ], F32, tag="lam")
            nhl = psmall.tile([P, NJ], F32, tag="nhl")
            xt_pss = []
            xty_pss = []
            for half in range(2):
                xt_ps = ppsx.tile([P, 512], F32, tag="xt_ps")
                xt_pss.append(xt_ps)
                xty_ps = ppsx.tile([P, 512], F32, tag="xt_ps")
                xty_pss.append(xty_ps)
            groups = [[0, 1, 2, 3], [4, 5, 6, 7]] if b == 0 else [list(range(NJ))]
            for grp in groups:
                g0, g1 = grp[0], grp[-1] + 1
                gc = slice(g0, g1)
                for j in grp:
                    sqjunk = pmain.tile([P, P], F32, tag="sqjunk")
                    nc.scalar.activation(
                        out=sqjunk[:],
                        in_=xgs[j][:],
                        func=mybir.ActivationFunctionType.Square,
                        accum_out=magsq[:, j : j + 1],
                    )
                nc.scalar.activation(out=mag[:, gc], in_=magsq[:, gc], func=Sqrt)
                nc.vector.reciprocal(out=rmag[:, gc], in_=mag[:, gc])
                nc.vector.tensor_tensor(
                    out=rmy[:, gc], in0=rmag[:, gc], in1=ycol_sb[:, gc], op=MULT
                )
                for j in grp:
                    nc.vector.tensor_scalar(
                        out=xhat[:, j, :],
                        in0=xgs[j][:],
                        scalar1=rmag[:, j : j + 1],
                        scalar2=None,
                        op0=MULT,
                    )
                    nc.scalar.mul(xhaty[:, j, :], xgs[j][:], rmy[:, j : j + 1])
                    half, k = divmod(j, 4)
                    nc.tensor.transpose(
                        out=xt_pss[half][:, k * P : (k + 1) * P],
                        in_=xhat[:, j, :],
                        identity=diag_sb[:],
                    )
                    nc.scalar.copy(
                        out=xhatT[:, j * P : (j + 1) * P],
                        in_=xt_pss[half][:, k * P : (k + 1) * P],
                    )
                    nc.tensor.transpose(
                        out=xty_pss[half][:, k * P : (k + 1) * P],
                        in_=xhaty[:, j, :],
                        identity=diag_sb[:],
                    )
                    nc.scalar.copy(
                        out=xhatTY[:, j * P : (j + 1) * P],
                        in_=xty_pss[half][:, k * P : (k + 1) * P],
                    )
                    nc.tensor.matmul(
                        out=lamdot_ps[:, j : j + 1],
                        lhsT=xhatT[:, j * P : (j + 1) * P].bitcast(F32),
                        rhs=lamw_sb[:],
                        start=True,
                        stop=True,
                    )
                nc.vector.tensor_tensor(
                    out=dotm[:, gc], in0=lamdot_ps[:, gc], in1=mag[:, gc], op=MULT
                )
                nc.scalar.activation(
                    out=lam[:, gc], in_=dotm[:, gc], func=Exp,
                    bias=lamb_sb[:], scale=1.0,
                )
                nc.vector.tensor_scalar(
                    out=nhl[:, gc], in0=lam[:, gc], scalar1=-0.5,
                    scalar2=None, op0=MULT,
                )
            ybc_sb = pbatch.tile([P, T], F32, tag="ybc_sb")
            nc.sync.dma_start(out=ybc_sb[:], in_=ybc[b])
            ybcr = pbatch.tile([P, T], MM_DTYPE, tag="ybcr")
            for half in range(2):
                nc.scalar.copy(
                    out=ybcr[:, half * 512 : (half + 1) * 512],
                    in_=ybc_sb[:, half * 512 : (half + 1) * 512],
                )

            wsum = psmall.tile([P, NJ], F32, tag="wsum")
            ynum = psmall.tile([P, NJ], F32, tag="ynum")

            # ---- main loop over query blocks ----
            for tb in range(NJ):
                W = (tb + 1) * P
                Woff = W - P
                nhalf = (W + 511) // 512
                q = pmain.tile([P, T], F32, tag="q")
                sw = pscan.tile([P, T], F32, tag="sw")
                sy = pscan.tile([P, T], F32, tag="sy")
                raw_pss = []
                rawy_pss = []
                widths = []
                for h in range(nhalf):
                    w0 = h * 512
                    wh = min(W, (h + 1) * 512) - w0
                    widths.append((w0, wh))
                    rp = pps.tile([P, 512], F32, tag="raw_ps")
                    raw_pss.append(rp)
                    ryp = ppsy.tile([P, 512], F32, tag="rawy_ps")
                    rawy_pss.append(ryp)
                    # rp  = xhat_t.xhat_s + 1        (inv128 @ ones = +1)
                    # ryp = xhat_t.xhat_s*y[s] + y[s]  (inv128 @ ybc = +y)
                    nc.tensor.matmul(
                        out=rp[:, :wh],
                        lhsT=xhatT[:, tb * P : (tb + 1) * P],
                        rhs=xhatT[:, w0 : w0 + wh],
                        start=True,
                        stop=False,
                    )
                    nc.tensor.matmul(
                        out=ryp[:, :wh],
                        lhsT=xhatT[:, tb * P : (tb + 1) * P],
                        rhs=xhatTY[:, w0 : w0 + wh],
                        start=True,
                        stop=False,
                    )
                    nc.tensor.matmul(
                        out=rp[:, :wh],
                        lhsT=inv_mm[:],
                        rhs=onesr[:, :wh],
                        start=False,
                        stop=True,
                    )
                    nc.tensor.matmul(
                        out=ryp[:, :wh],
                        lhsT=inv_mm[:],
                        rhs=ybcr[:, w0 : w0 + wh],
                        start=False,
                        stop=True,
                    )
                # diag block (last 128 cols): min-clamp in PSUM so masked
                # positions flow through the regular ACT ops as q=1, data0=0
                # -> scan state freezes at s=t-1 and the LAST scan column is
                # the answer for every row.
                hd = Woff // 512
                dl = Woff - hd * 512
                nc.vector.scalar_tensor_tensor(
                    out=raw_pss[hd][:, dl : dl + P],
                    in0=raw_pss[hd][:, dl : dl + P],
                    scalar=0.0,
                    in1=minfy_sb[:],
                    op0=ADD,
                    op1=MIN,
                )
                nc.vector.scalar_tensor_tensor(
                    out=rawy_pss[hd][:, dl : dl + P],
                    in0=rawy_pss[hd][:, dl : dl + P],
                    scalar=0.0,
                    in1=minfy_sb[:],
                    op0=ADD,
                    op1=MIN,
                )
                for h, (w0, wh) in enumerate(widths):
                    # q = exp(-lam/2 * (raw+1))
                    nc.scalar.activation(
                        out=q[:, w0 : w0 + wh],
                        in_=raw_pss[h][:, :wh],
                        func=Exp,
                        bias=0.0,
                        scale=nhl[:, tb : tb + 1],
                    )
                # scan: S[s] = (S[s-1] + data0[s]) * q[s]
                for h, (w0, wh) in enumerate(widths):
                    nc.vector.tensor_tensor_scan(
                        out=sw[:, w0 : w0 + wh],
                        data0=raw_pss[h][:, :wh],
                        data1=q[:, w0 : w0 + wh],
                        initial=0.0 if h == 0 else sw[:, w0 - 1 : w0],
                        op0=ADD,
                        op1=MULT,
                    )
                for h, (w0, wh) in enumerate(widths):
                    nc.vector.tensor_tensor_scan(
                        out=sy[:, w0 : w0 + wh],
                        data0=rawy_pss[h][:, :wh],
                        data1=q[:, w0 : w0 + wh],
                        initial=0.0 if h == 0 else sy[:, w0 - 1 : w0],
                        op0=ADD,
                        op1=MULT,
                    )
                nc.scalar.copy(out=wsum[:, tb : tb + 1], in_=sw[:, W - 1 : W])
                nc.scalar.copy(out=ynum[:, tb : tb + 1], in_=sy[:, W - 1 : W])

            # ---- finalize: yhat = clip(ynum / (wsum + 2e-6), .01, .99) ----
            wse = psmall.tile([P, NJ], F32, tag="wse")
            nc.vector.tensor_scalar(
                out=wse[:], in0=wsum[:], scalar1=2e-6, scalar2=None, op0=ADD
            )
            rcp = psmall.tile([P, NJ], F32, tag="rcp")
            nc.vector.reciprocal(out=rcp[:], in_=wse[:])
            yh = psmall.tile([P, NJ], F32, tag="yh")
            nc.vector.tensor_tensor(out=yh[:], in0=ynum[:], in1=rcp[:], op=MULT)
            yc = psmall.tile([P, NJ], F32, tag="yc")
            nc.vector.tensor_scalar(
                out=yc[:], in0=yh[:], scalar1=0.01, scalar2=0.99, op0=MAX, op1=MIN
            )
            nc.sync.dma_start(out=out[b], in_=yc[:])


def shard_inputs(y, problem_seq, embd_weight, lam_w, lam_b):
    """Build per-core input maps."""
    B = y.shape[0]
    assert B == N_CORES * NB_PER_CORE
    seq = np.ascontiguousarray(problem_seq).astype(np.int32)
    yf = np.ascontiguousarray(y).astype(np.float32)
    table = np.ascontiguousarray(embd_weight).astype(np.float32)
    lamw = np.ascontiguousarray(lam_w).reshape(P, 1).astype(np.float32)
    lamb = np.full((P, 1), np.float32(np.asarray(lam_b).reshape(-1)[0]))
    diag = np.eye(P, dtype=np.float32)
    smask = np.tril(np.ones((P, P), dtype=np.float32), k=-1)
    colv, rowv = np.meshgrid(np.arange(P), np.arange(P))
    minfw = np.where(colv < rowv, 1e30, -1.0).astype(np.float32)
    minfy = np.where(colv < rowv, 1e30, 0.0).astype(np.float32)

    in_maps = []
    for c in range(N_CORES):
        sl = slice(c * NB_PER_CORE, (c + 1) * NB_PER_CORE)
        # idx[b, p, j] = seq[b, j*128 + p]; ycol likewise
        idx = seq[sl].reshape(NB_PER_CORE, NJ, P).transpose(0, 2, 1)
        ycl = yf[sl].reshape(NB_PER_CORE, NJ, P).transpose(0, 2, 1)
        ybc_c = np.broadcast_to(yf[sl][:, None, :], (NB_PER_CORE, P, T))
        in_maps.append(
            {
                "table": table,
                "idx": np.ascontiguousarray(idx),
                "ybc": np.ascontiguousarray(ybc_c),
                "ycol": np.ascontiguousarray(ycl),
                "lamw": lamw,
                "lamb": lamb,
                "diag": diag,
                "smask": smask,
                "minfw": minfw,
                "minfy": minfy,
            }
        )
    return in_maps


def unshard_output(results):
    """results: list of 8 dicts with 'out' [4, 128, 8] -> yhat [32, 1024]."""
    parts = []
    for c in range(N_CORES):
        o = results[c]["out"]  # [NB, P, NJ]; yhat[b, j*128+p] = o[b, p, j]
        parts.append(o.transpose(0, 2, 1).reshape(NB_PER_CORE, T))
    return np.concatenate(parts, axis=0).astype(np.float32)


_NC_CACHE = None


def _get_program():
    global _NC_CACHE
    if _NC_CACHE is None:
        _NC_CACHE = build_program()
    return _NC_CACHE


def kernel(y, problem_seq, embd_weight, lam_w, lam_b, _trace=False, **trace_kwargs):
    nc = _get_program()
    in_maps = shard_inputs(y, problem_seq, embd_weight, lam_w, lam_b)
    res = run_bass_kernel_spmd(
        nc, in_maps, core_ids=list(range(N_CORES)), trace=_trace, **trace_kwargs
    )
    outp = unshard_output(res.results)
    if _trace:
        return outp, res
    return outp


if __name__ == "__main__":
    rng = np.random.default_rng(0)
    y = rng.random((32, T), dtype=np.float32)
    seq = rng.integers(0, N_VOCAB, size=(32, T)).astype(np.int32)
    emb = rng.standard_normal((N_VOCAB, P), dtype=np.float32)
    lw = (rng.standard_normal((P, 1), dtype=np.float32) / np.sqrt(P)).astype(np.float32)
    lb = (rng.standard_normal((1,), dtype=np.float32) * 0.01).astype(np.float32)
    outp = kernel(y, seq, emb, lw, lb)
    print("out", outp.shape, outp.dtype, outp[:2, :5])


# revision 42
# speedup vs baseline: 1.0355x; 1.0355x over previous
"""Bass/Trainium2 kernel for nn_ExpMovAvgModel (sparse_attention).

Math (per batch row b, query t, key s, H=128 hidden):
    x      = embd[seq]                        # [T, H] gathered rows
    xhat   = x / |x|                          # row-normalized
    raw    = xhat @ xhat.T                    # cosine similarity [T, T]
    sim01  = 0.5*(raw+1) masked to s < t
    delta  = reversed-cumsum_s(sim01)         # = sum_{v=s}^{t-1} sim01[v]
    lam    = exp(x @ lam_w + lam_b)
    w      = sim01 * exp(-lam*delta)
    yhat   = clip((w @ y) / (sum_s w + 1e-6), 0.01, 0.99)

Key restructure: with q[u] = exp(-lam*sim01[u]), the forward scan
    S[s] = (S[s-1] + (raw[u]+1)) * q[s]
satisfies S[t-1] = 2*sum_s w[t,s]; with data0 = (raw[u]+1)*y[u] it gives
2*(w @ y).  One tensor_tensor_scan per row-block replaces the masked
reversed cumsum, the [T,T] exp, the weight multiply and the row
reduction.  The strict-causal mask is applied only to the 128-wide
diagonal block, and masked positions get data0=0, q=exp(0)=1 so the
scan state FREEZES at s=t-1: the last scan column is the answer for
every row (no per-row diagonal extraction).

Engine placement (measured): scans + diag masks on DVE; exp on ACT; all
(raw+1) / (raw+1)*y tiles are built by the PE itself - the sim matmul,
a second matmul against y-scaled embeddings, and K=1 all-ones matmuls
that add +1 / +y[s] into PSUM - so the scans read PSUM directly and DVE
does almost nothing but scan.  GPSIMD only runs the embedding gather
(its compute throughput is terrible).  Matmuls run in float32r
(1 cycle/row at N>=256 vs 4 for fp32; ~2e-3 rel err, well in budget).

Sharding: data-parallel over batch B=32 -> 4 batches per core x 8 cores.
"""

import os
import sys

import numpy as np

for _p in ("/opt/trn_rl_repo",):
    if _p not in sys.path and os.path.isdir(_p):
        sys.path.append(_p)

import concourse.bass as bass
import concourse.tile as tile
from concourse import bacc, mybir
from concourse.bass_utils import run_bass_kernel_spmd

P = 128            # partitions / hidden dim
T = 1024           # sequence length
NJ = T // P        # 8 column-blocks
NB_PER_CORE = 4    # batches per core
N_CORES = 8
N_VOCAB = 50000

F32 = mybir.dt.float32
I32 = mybir.dt.int32
MM_DTYPE = mybir.dt.float32r


def build_program():
    nc = bacc.Bacc(
        "TRN2",
        target_bir_lowering=False,
        debug=False,
        num_devices=N_CORES,
    )

    table = nc.dram_tensor("table", [N_VOCAB, P], F32, kind="ExternalInput").ap()
    idx = nc.dram_tensor("idx", [NB_PER_CORE, P, NJ], I32, kind="ExternalInput").ap()
    ybc = nc.dram_tensor("ybc", [NB_PER_CORE, P, T], F32, kind="ExternalInput").ap()
    ycol = nc.dram_tensor("ycol", [NB_PER_CORE, P, NJ], F32, kind="ExternalInput").ap()
    lamw = nc.dram_tensor("lamw", [P, 1], F32, kind="ExternalInput").ap()
    lamb = nc.dram_tensor("lamb", [P, 1], F32, kind="ExternalInput").ap()
    diag = nc.dram_tensor("diag", [P, P], F32, kind="ExternalInput").ap()
    smask = nc.dram_tensor("smask", [P, P], F32, kind="ExternalInput").ap()
    minfw = nc.dram_tensor("minfw", [P, P], F32, kind="ExternalInput").ap()
    minfy = nc.dram_tensor("minfy", [P, P], F32, kind="ExternalInput").ap()
    out = nc.dram_tensor("out", [NB_PER_CORE, P, NJ], F32, kind="ExternalOutput").ap()

    with tile.TileContext(nc) as tc:
        _build_body(tc, table, idx, ybc, ycol, lamw, lamb, diag, smask, minfw, minfy, out)

    nc.compile()
    return nc


def _build_body(tc, table, idx, ybc, ycol, lamw, lamb, diag, smask, minfw, minfy, out):
    from contextlib import ExitStack

    nc = tc.nc
    Exp = mybir.ActivationFunctionType.Exp
    Sqrt = mybir.ActivationFunctionType.Sqrt
    ADD = mybir.AluOpType.add
    MULT = mybir.AluOpType.mult
    MAX = mybir.AluOpType.max
    MIN = mybir.AluOpType.min

    with ExitStack() as ctx:
        pconst = ctx.enter_context(tc.tile_pool(name="pconst", bufs=1))
        pbatch = ctx.enter_context(tc.tile_pool(name="pbatch", bufs=2))
        psmall = ctx.enter_context(tc.tile_pool(name="psmall", bufs=2))
        pmain = ctx.enter_context(tc.tile_pool(name="pmain", bufs=3))
        pscan = ctx.enter_context(tc.tile_pool(name="pscan", bufs=5))
        pps = ctx.enter_context(tc.tile_pool(name="pps", bufs=3, space="PSUM"))
        ppsy = ctx.enter_context(tc.tile_pool(name="ppsy", bufs=2, space="PSUM"))
        ppsx = ctx.enter_context(tc.tile_pool(name="ppsx", bufs=2, space="PSUM"))
        ppsl = ctx.enter_context(tc.tile_pool(name="ppsl", bufs=1, space="PSUM"))

        diag_sb = pconst.tile([P, P], F32)
        nc.sync.dma_start(out=diag_sb[:], in_=diag)
        lamw_sb = pconst.tile([P, 1], F32)
        nc.sync.dma_start(out=lamw_sb[:], in_=lamw)
        lamb_sb = pconst.tile([P, 1], F32)
        nc.sync.dma_start(out=lamb_sb[:], in_=lamb)
        smask_sb = pconst.tile([P, P], F32)
        nc.sync.dma_start(out=smask_sb[:], in_=smask)
        # min-masks: clamping the diag PSUM block with these makes the masked
        # positions produce exactly the frozen-scan values through the regular
        # ACT ops: raw=-1 -> q=exp(0)=1, rawp1=0; rawy=0 (valid since y>0)
        minfw_sb = pconst.tile([P, P], F32)
        nc.sync.dma_start(out=minfw_sb[:], in_=minfw)
        minfy_sb = pconst.tile([P, P], F32)
        nc.sync.dma_start(out=minfy_sb[:], in_=minfy)
        # all-(1/128) matrix: K=128 matmul against a broadcast y tile adds
        # +y[s] to every row of a PSUM at full PE rate (K=1 matmuls are slow)
        inv_f32 = pconst.tile([P, P], F32)
        nc.vector.memset(inv_f32[:], 1.0 / P)
        inv_mm = pconst.tile([P, P], MM_DTYPE)
        nc.scalar.copy(out=inv_mm[:], in_=inv_f32[:])
        onesr_f32 = pconst.tile([P, 512], F32)
        nc.vector.memset(onesr_f32[:], 1.0)
        onesr = pconst.tile([P, 512], MM_DTYPE)
        nc.scalar.copy(out=onesr[:], in_=onesr_f32[:])

        for b in range(NB_PER_CORE):
            # ---- gather x rows: xg_j[p, :] = table[seq[j*128+p]] ----
            # (HW indirect DMA: one index per partition, contiguous out)
            idx_sb = psmall.tile([P, NJ], I32, tag="idx_sb")
            nc.sync.dma_start(out=idx_sb[:], in_=idx[b])
            xgs = []
            for j in range(NJ):
                xg = pbatch.tile([P, P], F32, tag=f"xg{j}")
                nc.gpsimd.indirect_dma_start(
                    out=xg[:],
                    out_offset=None,
                    in_=table,
                    in_offset=bass.IndirectOffsetOnAxis(
                        ap=idx_sb[:, j : j + 1], axis=0
                    ),
                )
                xgs.append(xg)

            # ---- norms, normalize, transpose, lam ----
            # batch 0 runs a per-j pipeline (tb=0 can start right after the
            # first gather -> kills the cold-start stall); later batches use
            # batched ops (fewer ACT table switches / small-op overheads)
            magsq = psmall.tile([P, NJ], F32, tag="magsq")
            mag = psmall.tile([P, NJ], F32, tag="mag")
            rmag = psmall.tile([P, NJ], F32, tag="rmag")
            xhat = pbatch.tile([P, NJ, P], F32, tag="xhat")
            xhatT = pbatch.tile([P, T], MM_DTYPE, tag="xhatT")
            lamdot_ps = ppsl.tile([P, NJ], F32, tag="lamdot_ps")
            dotm = psmall.tile([P, NJ], F32, tag="dotm")
            lam = psmall.tile([P, NJ], F32, tag="lam")
            nhl = psmall.tile([P, NJ], F32, tag="nhl")
            xt_pss = []
            for half in range(2):
                xt_ps = ppsx.tile([P, 512], F32, tag="xt_ps")
                xt_pss.append(xt_ps)
            groups = [[0, 1, 2, 3], [4, 5, 6, 7]] if b == 0 else [list(range(NJ))]
            for grp in groups:
                g0, g1 = grp[0], grp[-1] + 1
                gc = slice(g0, g1)
                for j in grp:
                    sqjunk = pmain.tile([P, P], F32, tag="sqjunk")
                    nc.scalar.activation(
                        out=sqjunk[:],
                        in_=xgs[j][:],
                        func=mybir.ActivationFunctionType.Square,
                        accum_out=magsq[:, j : j + 1],
                    )
                nc.scalar.activation(out=mag[:, gc], in_=magsq[:, gc], func=Sqrt)
                nc.vector.reciprocal(out=rmag[:, gc], in_=mag[:, gc])
                for j in grp:
                    nc.vector.tensor_scalar(
                        out=xhat[:, j, :],
                        in0=xgs[j][:],
                        scalar1=rmag[:, j : j + 1],
                        scalar2=None,
                        op0=MULT,
                    )
                    half, k = divmod(j, 4)
                    nc.tensor.transpose(
                        out=xt_pss[half][:, k * P : (k + 1) * P],
                        in_=xhat[:, j, :],
                        identity=diag_sb[:],
                    )
                    nc.scalar.copy(
                        out=xhatT[:, j * P : (j + 1) * P],
                        in_=xt_pss[half][:, k * P : (k + 1) * P],
                    )
                    nc.tensor.matmul(
                        out=lamdot_ps[:, j : j + 1],
                        lhsT=xhatT[:, j * P : (j + 1) * P].bitcast(F32),
                        rhs=lamw_sb[:],
                        start=True,
                        stop=True,
                    )
                nc.vector.tensor_tensor(
                    out=dotm[:, gc], in0=lamdot_ps[:, gc], in1=mag[:, gc], op=MULT
                )
                nc.scalar.activation(
                    out=lam[:, gc], in_=dotm[:, gc], func=Exp,
                    bias=lamb_sb[:], scale=1.0,
                )
                nc.vector.tensor_scalar(
                    out=nhl[:, gc], in0=lam[:, gc], scalar1=-0.5,
                    scalar2=None, op0=MULT,
                )
            ybc_sb = pbatch.tile([P, T], F32, tag="ybc_sb")
            nc.sync.dma_start(out=ybc_sb[:], in_=ybc[b])
            ybcr = pbatch.tile([P, T], MM_DTYPE, tag="ybcr")
            for half in range(2):
                nc.scalar.copy(
                    out=ybcr[:, half * 512 : (half + 1) * 512],
                    in_=ybc_sb[:, half * 512 : (half + 1) * 512],
                )
            xhatTY = pbatch.tile([P, T], MM_DTYPE, tag="xhatTY")
            nc.vector.tensor_tensor(
                out=xhatTY[:], in0=xhatT[:].bitcast(F32), in1=ybc_sb[:], op=MULT
            )

            wsum = psmall.tile([P, NJ], F32, tag="wsum")
            ynum = psmall.tile([P, NJ], F32, tag="ynum")

            # ---- main loop over query blocks ----
            for tb in range(NJ):
                W = (tb + 1) * P
                Woff = W - P
                nhalf = (W + 511) // 512
                q = pmain.tile([P, T], F32, tag="q")
                sw = pscan.tile([P, T], F32, tag="sw")
                sy = pscan.tile([P, T], F32, tag="sy")
                raw_pss = []
                rawy_pss = []
                widths = []
                for h in range(nhalf):
                    w0 = h * 512
                    wh = min(W, (h + 1) * 512) - w0
                    widths.append((w0, wh))
                    rp = pps.tile([P, 512], F32, tag="raw_ps")
                    raw_pss.append(rp)
                    ryp = ppsy.tile([P, 512], F32, tag="rawy_ps")
                    rawy_pss.append(ryp)
                    # rp  = xhat_t.xhat_s + 1        (inv128 @ ones = +1)
                    # ryp = xhat_t.xhat_s*y[s] + y[s]  (inv128 @ ybc = +y)
                    nc.tensor.matmul(
                        out=rp[:, :wh],
                        lhsT=xhatT[:, tb * P : (tb + 1) * P],
                        rhs=xhatT[:, w0 : w0 + wh],
                        start=True,
                        stop=False,
                    )
                    nc.tensor.matmul(
                        out=ryp[:, :wh],
                        lhsT=xhatT[:, tb * P : (tb + 1) * P],
                        rhs=xhatTY[:, w0 : w0 + wh],
                        start=True,
                        stop=False,
                    )
                    nc.tensor.matmul(
                        out=rp[:, :wh],
                        lhsT=inv_mm[:],
                        rhs=onesr[:, :wh],
                        start=False,
                        stop=True,
                    )
                    nc.tensor.matmul(
                        out=ryp[:, :wh],
                        lhsT=inv_mm[:],
                        rhs=ybcr[:, w0 : w0 + wh],
                        start=False,
                        stop=True,
                    )
                # diag block (last 128 cols): min-clamp in PSUM so masked
                # positions flow through the regular ACT ops as q=1, data0=0
                # -> scan state freezes at s=t-1 and the LAST scan column is
                # the answer for every row.
                hd = Woff // 512
                dl = Woff - hd * 512
                nc.vector.scalar_tensor_tensor(
                    out=raw_pss[hd][:, dl : dl + P],
                    in0=raw_pss[hd][:, dl : dl + P],
                    scalar=0.0,
                    in1=minfy_sb[:],
                    op0=ADD,
                    op1=MIN,
                )
                nc.vector.scalar_tensor_tensor(
                    out=rawy_pss[hd][:, dl : dl + P],
                    in0=rawy_pss[hd][:, dl : dl + P],
                    scalar=0.0,
                    in1=minfy_sb[:],
                    op0=ADD,
                    op1=MIN,
                )
                for h, (w0, wh) in enumerate(widths):
                    # q = exp(-lam/2 * (raw+1))
                    nc.scalar.activation(
                        out=q[:, w0 : w0 + wh],
                        in_=raw_pss[h][:, :wh],
                        func=Exp,
                        bias=0.0,
                        scale=nhl[:, tb : tb + 1],
                    )
                # scan: S[s] = (S[s-1] + data0[s]) * q[s]
                for h, (w0, wh) in enumerate(widths):
                    nc.vector.tensor_tensor_scan(
                        out=sw[:, w0 : w0 + wh],
                        data0=raw_pss[h][:, :wh],
                        data1=q[:, w0 : w0 + wh],
                        initial=0.0 if h == 0 else sw[:, w0 - 1 : w0],
                        op0=ADD,
                        op1=MULT,
                    )
                for h, (w0, wh) in enumerate(widths):
                    nc.vector.tensor_tensor_scan(
                        out=sy[:, w0 : w0 + wh],
                        data0=rawy_pss[h][:, :wh],
                        data1=q[:, w0 : w0 + wh],
                        initial=0.0 if h == 0 else sy[:, w0 - 1 : w0],
                        op0=ADD,
                        op1=MULT,
                    )
                nc.scalar.copy(out=wsum[:, tb : tb + 1], in_=sw[:, W - 1 : W])
                nc.scalar.copy(out=ynum[:, tb : tb + 1], in_=sy[:, W - 1 : W])

            # ---- finalize: yhat = clip(ynum / (wsum + 2e-6), .01, .99) ----
            wse = psmall.tile([P, NJ], F32, tag="wse")
            nc.vector.tensor_scalar(
                out=wse[:], in0=wsum[:], scalar1=2e-6, scalar2=None, op0=ADD
            )
            rcp = psmall.tile([P, NJ], F32, tag="rcp")
            nc.vector.reciprocal(out=rcp[:], in_=wse[:])
            yh = psmall.tile([P, NJ], F32, tag="yh")
            nc.vector.tensor_tensor(out=yh[:], in0=ynum[:], in1=rcp[:], op=MULT)
            yc = psmall.tile([P, NJ], F32, tag="yc")
            nc.vector.tensor_scalar(
                out=yc[:], in0=yh[:], scalar1=0.01, scalar2=0.99, op0=MAX, op1=MIN
            )
            nc.sync.dma_start(out=out[b], in_=yc[:])


def shard_inputs(y, problem_seq, embd_weight, lam_w, lam_b):
    """Build per-core input maps."""
    B = y.shape[0]
    assert B == N_CORES * NB_PER_CORE
    seq = np.ascontiguousarray(problem_seq).astype(np.int32)
    yf = np.ascontiguousarray(y).astype(np.float32)
    table = np.ascontiguousarray(embd_weight).astype(np.float32)
    lamw = np.ascontiguousarray(lam_w).reshape(P, 1).astype(np.float32)
    lamb = np.full((P, 1), np.float32(np.asarray(lam_b).reshape(-1)[0]))
    diag = np.eye(P, dtype=np.float32)
    smask = np.tril(np.ones((P, P), dtype=np.float32), k=-1)
    colv, rowv = np.meshgrid(np.arange(P), np.arange(P))
    minfw = np.where(colv < rowv, 1e30, -1.0).astype(np.float32)
    minfy = np.where(colv < rowv, 1e30, 0.0).astype(np.float32)

    in_maps = []
    for c in range(N_CORES):
        sl = slice(c * NB_PER_CORE, (c + 1) * NB_PER_CORE)
        # idx[b, p, j] = seq[b, j*128 + p]; ycol likewise
        idx = seq[sl].reshape(NB_PER_CORE, NJ, P).transpose(0, 2, 1)
        ycl = yf[sl].reshape(NB_PER_CORE, NJ, P).transpose(0, 2, 1)
        ybc_c = np.broadcast_to(yf[sl][:, None, :], (NB_PER_CORE, P, T))
        in_maps.append(
            {
                "table": table,
                "idx": np.ascontiguousarray(idx),
                "ybc": np.ascontiguousarray(ybc_c),
                "ycol": np.ascontiguousarray(ycl),
                "lamw": lamw,
                "lamb": lamb,
                "diag": diag,
                "smask": smask,
                "minfw": minfw,
                "minfy": minfy,
            }
        )
    return in_maps


def unshard_output(results):
    """results: list of 8 dicts with 'out' [4, 128, 8] -> yhat [32, 1024]."""
    parts = []
    for c in range(N_CORES):
        o = results[c]["out"]  # [NB, P, NJ]; yhat[b, j*128+p] = o[b, p, j]
        parts.append(o.transpose(0, 2, 1).reshape(NB_PER_CORE, T))
    return np.concatenate(parts, axis=0).astype(np.float32)


_NC_CACHE = None


def _get_program():
    global _NC_CACHE
    if _NC_CACHE is None:
        _NC_CACHE = build_program()
    return _NC_CACHE


def kernel(y, problem_seq, embd_weight, lam_w, lam_b, _trace=False, **trace_kwargs):
    nc = _get_program()
    in_maps = shard_inputs(y, problem_seq, embd_weight, lam_w, lam_b)
    res = run_bass_kernel_spmd(
        nc, in_maps, core_ids=list(range(N_CORES)), trace=_trace, **trace_kwargs
    )
    outp = unshard_output(res.results)
    if _trace:
        return outp, res
    return outp


if __name__ == "__main__":
    rng = np.random.default_rng(0)
    y = rng.random((32, T), dtype=np.float32)
    seq = rng.integers(0, N_VOCAB, size=(32, T)).astype(np.int32)
    emb = rng.standard_normal((N_VOCAB, P), dtype=np.float32)
    lw = (rng.standard_normal((P, 1), dtype=np.float32) / np.sqrt(P)).astype(np.float32)
    lb = (rng.standard_normal((1,), dtype=np.float32) * 0.01).astype(np.float32)
    outp = kernel(y, seq, emb, lw, lb)
    print("out", outp.shape, outp.dtype, outp[:2, :5])


# revision 43
# speedup vs baseline: 1.0815x; 1.0444x over previous
"""Bass/Trainium2 kernel for nn_ExpMovAvgModel (sparse_attention).

Math (per batch row b, query t, key s, H=128 hidden):
    x      = embd[seq]                        # [T, H] gathered rows
    xhat   = x / |x|                          # row-normalized
    raw    = xhat @ xhat.T                    # cosine similarity [T, T]
    sim01  = 0.5*(raw+1) masked to s < t
    delta  = reversed-cumsum_s(sim01)         # = sum_{v=s}^{t-1} sim01[v]
    lam    = exp(x @ lam_w + lam_b)
    w      = sim01 * exp(-lam*delta)
    yhat   = clip((w @ y) / (sum_s w + 1e-6), 0.01, 0.99)

Key restructure: with q[u] = exp(-lam*sim01[u]), the forward scan
    S[s] = (S[s-1] + (raw[u]+1)) * q[s]
satisfies S[t-1] = 2*sum_s w[t,s]; with data0 = (raw[u]+1)*y[u] it gives
2*(w @ y).  One tensor_tensor_scan per row-block replaces the masked
reversed cumsum, the [T,T] exp, the weight multiply and the row
reduction.  The strict-causal mask is applied only to the 128-wide
diagonal block, and masked positions get data0=0, q=exp(0)=1 so the
scan state FREEZES at s=t-1: the last scan column is the answer for
every row (no per-row diagonal extraction).

Engine placement (measured): scans + diag masks on DVE; exp on ACT; all
(raw+1) / (raw+1)*y tiles are built by the PE itself - the sim matmul,
a second matmul against y-scaled embeddings, and K=1 all-ones matmuls
that add +1 / +y[s] into PSUM - so the scans read PSUM directly and DVE
does almost nothing but scan.  GPSIMD only runs the embedding gather
(its compute throughput is terrible).  Matmuls run in float32r
(1 cycle/row at N>=256 vs 4 for fp32; ~2e-3 rel err, well in budget).

Sharding: data-parallel over batch B=32 -> 4 batches per core x 8 cores.
"""

import os
import sys

import numpy as np

for _p in ("/opt/trn_rl_repo",):
    if _p not in sys.path and os.path.isdir(_p):
        sys.path.append(_p)

import concourse.bass as bass
import concourse.tile as tile
from concourse import bacc, mybir
from concourse.bass_utils import run_bass_kernel_spmd

P = 128            # partitions / hidden dim
T = 1024           # sequence length
NJ = T // P        # 8 column-blocks
NB_PER_CORE = 4    # batches per core
N_CORES = 8
N_VOCAB = 50000

F32 = mybir.dt.float32
I32 = mybir.dt.int32
MM_DTYPE = mybir.dt.float32r


def build_program():
    nc = bacc.Bacc(
        "TRN2",
        target_bir_lowering=False,
        debug=False,
        num_devices=N_CORES,
    )

    table = nc.dram_tensor("table", [N_VOCAB, P], F32, kind="ExternalInput").ap()
    idx = nc.dram_tensor("idx", [NB_PER_CORE, P, NJ], I32, kind="ExternalInput").ap()
    ybc = nc.dram_tensor("ybc", [NB_PER_CORE, P, T], F32, kind="ExternalInput").ap()
    ycol = nc.dram_tensor("ycol", [NB_PER_CORE, P, NJ], F32, kind="ExternalInput").ap()
    lamw = nc.dram_tensor("lamw", [P, 1], F32, kind="ExternalInput").ap()
    lamb = nc.dram_tensor("lamb", [P, 1], F32, kind="ExternalInput").ap()
    diag = nc.dram_tensor("diag", [P, P], F32, kind="ExternalInput").ap()
    smask = nc.dram_tensor("smask", [P, P], F32, kind="ExternalInput").ap()
    minfw = nc.dram_tensor("minfw", [P, P], F32, kind="ExternalInput").ap()
    minfy = nc.dram_tensor("minfy", [P, P], F32, kind="ExternalInput").ap()
    out = nc.dram_tensor("out", [NB_PER_CORE, P, NJ], F32, kind="ExternalOutput").ap()

    with tile.TileContext(nc) as tc:
        _build_body(tc, table, idx, ybc, ycol, lamw, lamb, diag, smask, minfw, minfy, out)

    nc.compile()
    return nc


def _build_body(tc, table, idx, ybc, ycol, lamw, lamb, diag, smask, minfw, minfy, out):
    from contextlib import ExitStack

    nc = tc.nc
    Exp = mybir.ActivationFunctionType.Exp
    Sqrt = mybir.ActivationFunctionType.Sqrt
    ADD = mybir.AluOpType.add
    MULT = mybir.AluOpType.mult
    MAX = mybir.AluOpType.max
    MIN = mybir.AluOpType.min

    with ExitStack() as ctx:
        pconst = ctx.enter_context(tc.tile_pool(name="pconst", bufs=1))
        pbatch = ctx.enter_context(tc.tile_pool(name="pbatch", bufs=2))
        psmall = ctx.enter_context(tc.tile_pool(name="psmall", bufs=2))
        pmain = ctx.enter_context(tc.tile_pool(name="pmain", bufs=3))
        pscan = ctx.enter_context(tc.tile_pool(name="pscan", bufs=5))
        pps = ctx.enter_context(tc.tile_pool(name="pps", bufs=3, space="PSUM"))
        ppsx = ctx.enter_context(tc.tile_pool(name="ppsx", bufs=1, space="PSUM"))
        ppsl = ctx.enter_context(tc.tile_pool(name="ppsl", bufs=1, space="PSUM"))

        diag_sb = pconst.tile([P, P], F32)
        nc.sync.dma_start(out=diag_sb[:], in_=diag)
        lamw_sb = pconst.tile([P, 1], F32)
        nc.sync.dma_start(out=lamw_sb[:], in_=lamw)
        lamb_sb = pconst.tile([P, 1], F32)
        nc.sync.dma_start(out=lamb_sb[:], in_=lamb)
        smask_sb = pconst.tile([P, P], F32)
        nc.sync.dma_start(out=smask_sb[:], in_=smask)
        # min-masks: clamping the diag PSUM block with these makes the masked
        # positions produce exactly the frozen-scan values through the regular
        # ACT ops: raw=-1 -> q=exp(0)=1, rawp1=0; rawy=0 (valid since y>0)
        minfw_sb = pconst.tile([P, P], F32)
        nc.sync.dma_start(out=minfw_sb[:], in_=minfw)
        minfy_sb = pconst.tile([P, P], F32)
        nc.sync.dma_start(out=minfy_sb[:], in_=minfy)
        # all-(1/128) matrix: K=128 matmul against a broadcast y tile adds
        # +y[s] to every row of a PSUM at full PE rate (K=1 matmuls are slow)
        inv_f32 = pconst.tile([P, P], F32)
        nc.vector.memset(inv_f32[:], 1.0 / P)
        inv_mm = pconst.tile([P, P], MM_DTYPE)
        nc.scalar.copy(out=inv_mm[:], in_=inv_f32[:])
        onesr_f32 = pconst.tile([P, 512], F32)
        nc.vector.memset(onesr_f32[:], 1.0)
        onesr = pconst.tile([P, 512], MM_DTYPE)
        nc.scalar.copy(out=onesr[:], in_=onesr_f32[:])

        for b in range(NB_PER_CORE):
            # ---- gather x rows: xg_j[p, :] = table[seq[j*128+p]] ----
            # (HW indirect DMA: one index per partition, contiguous out)
            idx_sb = psmall.tile([P, NJ], I32, tag="idx_sb")
            nc.sync.dma_start(out=idx_sb[:], in_=idx[b])
            xgs = []
            for j in range(NJ):
                xg = pbatch.tile([P, P], F32, tag=f"xg{j}")
                nc.gpsimd.indirect_dma_start(
                    out=xg[:],
                    out_offset=None,
                    in_=table,
                    in_offset=bass.IndirectOffsetOnAxis(
                        ap=idx_sb[:, j : j + 1], axis=0
                    ),
                )
                xgs.append(xg)

            # ---- norms, normalize, transpose, lam ----
            # batch 0 runs a per-j pipeline (tb=0 can start right after the
            # first gather -> kills the cold-start stall); later batches use
            # batched ops (fewer ACT table switches / small-op overheads)
            magsq = psmall.tile([P, NJ], F32, tag="magsq")
            mag = psmall.tile([P, NJ], F32, tag="mag")
            rmag = psmall.tile([P, NJ], F32, tag="rmag")
            xhat = pbatch.tile([P, NJ, P], F32, tag="xhat")
            xhatT = pbatch.tile([P, T], MM_DTYPE, tag="xhatT")
            lamdot_ps = ppsl.tile([P, NJ], F32, tag="lamdot_ps")
            dotm = psmall.tile([P, NJ], F32, tag="dotm")
            lam = psmall.tile([P, NJ], F32, tag="lam")
            nhl = psmall.tile([P, NJ], F32, tag="nhl")
            xt_pss = []
            for half in range(2):
                xt_ps = ppsx.tile([P, 512], F32, tag="xt_ps")
                xt_pss.append(xt_ps)
            groups = [[0, 1, 2, 3], [4, 5, 6, 7]] if b == 0 else [list(range(NJ))]
            for grp in groups:
                g0, g1 = grp[0], grp[-1] + 1
                gc = slice(g0, g1)
                for j in grp:
                    sqjunk = pmain.tile([P, P], F32, tag="sqjunk")
                    nc.scalar.activation(
                        out=sqjunk[:],
                        in_=xgs[j][:],
                        func=mybir.ActivationFunctionType.Square,
                        accum_out=magsq[:, j : j + 1],
                    )
                nc.scalar.activation(out=mag[:, gc], in_=magsq[:, gc], func=Sqrt)
                nc.vector.reciprocal(out=rmag[:, gc], in_=mag[:, gc])
                for j in grp:
                    nc.vector.tensor_scalar(
                        out=xhat[:, j, :],
                        in0=xgs[j][:],
                        scalar1=rmag[:, j : j + 1],
                        scalar2=None,
                        op0=MULT,
                    )
                    half, k = divmod(j, 4)
                    nc.tensor.transpose(
                        out=xt_pss[half][:, k * P : (k + 1) * P],
                        in_=xhat[:, j, :],
                        identity=diag_sb[:],
                    )
                    nc.scalar.copy(
                        out=xhatT[:, j * P : (j + 1) * P],
                        in_=xt_pss[half][:, k * P : (k + 1) * P],
                    )
                    nc.tensor.matmul(
                        out=lamdot_ps[:, j : j + 1],
                        lhsT=xhatT[:, j * P : (j + 1) * P].bitcast(F32),
                        rhs=lamw_sb[:],
                        start=True,
                        stop=True,
                    )
                nc.vector.tensor_tensor(
                    out=dotm[:, gc], in0=lamdot_ps[:, gc], in1=mag[:, gc], op=MULT
                )
                nc.scalar.activation(
                    out=lam[:, gc], in_=dotm[:, gc], func=Exp,
                    bias=lamb_sb[:], scale=1.0,
                )
                nc.vector.tensor_scalar(
                    out=nhl[:, gc], in0=lam[:, gc], scalar1=-0.5,
                    scalar2=None, op0=MULT,
                )
            ybc_sb = pbatch.tile([P, T], F32, tag="ybc_sb")
            nc.sync.dma_start(out=ybc_sb[:], in_=ybc[b])
            ybcr = pbatch.tile([P, T], MM_DTYPE, tag="ybcr")
            for half in range(2):
                nc.scalar.copy(
                    out=ybcr[:, half * 512 : (half + 1) * 512],
                    in_=ybc_sb[:, half * 512 : (half + 1) * 512],
                )
            xhatTY = pbatch.tile([P, T], MM_DTYPE, tag="xhatTY")
            nc.vector.tensor_tensor(
                out=xhatTY[:], in0=xhatT[:].bitcast(F32), in1=ybc_sb[:], op=MULT
            )

            wsum = psmall.tile([P, NJ], F32, tag="wsum")
            ynum = psmall.tile([P, NJ], F32, tag="ynum")

            # ---- main loop over query blocks ----
            for tb in range(NJ):
                W = (tb + 1) * P
                Woff = W - P
                nhalf = (W + 511) // 512
                q = pmain.tile([P, T], F32, tag="q")
                sw = pscan.tile([P, T], F32, tag="sw")
                sy = pscan.tile([P, T], F32, tag="sy")
                raw_pss = []
                rawy_pss = []
                widths = []
                for h in range(nhalf):
                    w0 = h * 512
                    wh = min(W, (h + 1) * 512) - w0
                    widths.append((w0, wh))
                    rp = pps.tile([P, 512], F32, tag="raw_ps")
                    raw_pss.append(rp)
                    ryp = pps.tile([P, 512], F32, tag="rawy_ps")
                    rawy_pss.append(ryp)
                    # rp  = xhat_t.xhat_s + 1        (inv128 @ ones = +1)
                    # ryp = xhat_t.xhat_s*y[s] + y[s]  (inv128 @ ybc = +y)
                    nc.tensor.matmul(
                        out=rp[:, :wh],
                        lhsT=xhatT[:, tb * P : (tb + 1) * P],
                        rhs=xhatT[:, w0 : w0 + wh],
                        start=True,
                        stop=False,
                    )
                    nc.tensor.matmul(
                        out=ryp[:, :wh],
                        lhsT=xhatT[:, tb * P : (tb + 1) * P],
                        rhs=xhatTY[:, w0 : w0 + wh],
                        start=True,
                        stop=False,
                    )
                    nc.tensor.matmul(
                        out=rp[:, :wh],
                        lhsT=inv_mm[:],
                        rhs=onesr[:, :wh],
                        start=False,
                        stop=True,
                    )
                    nc.tensor.matmul(
                        out=ryp[:, :wh],
                        lhsT=inv_mm[:],
                        rhs=ybcr[:, w0 : w0 + wh],
                        start=False,
                        stop=True,
                    )
                # diag block (last 128 cols): min-clamp in PSUM so masked
                # positions flow through the regular ACT ops as q=1, data0=0
                # -> scan state freezes at s=t-1 and the LAST scan column is
                # the answer for every row.
                hd = Woff // 512
                dl = Woff - hd * 512
                nc.vector.scalar_tensor_tensor(
                    out=raw_pss[hd][:, dl : dl + P],
                    in0=raw_pss[hd][:, dl : dl + P],
                    scalar=0.0,
                    in1=minfy_sb[:],
                    op0=ADD,
                    op1=MIN,
                )
                nc.vector.scalar_tensor_tensor(
                    out=rawy_pss[hd][:, dl : dl + P],
                    in0=rawy_pss[hd][:, dl : dl + P],
                    scalar=0.0,
                    in1=minfy_sb[:],
                    op0=ADD,
                    op1=MIN,
                )
                for h, (w0, wh) in enumerate(widths):
                    # q = exp(-lam/2 * (raw+1))
                    nc.scalar.activation(
                        out=q[:, w0 : w0 + wh],
                        in_=raw_pss[h][:, :wh],
                        func=Exp,
                        bias=0.0,
                        scale=nhl[:, tb : tb + 1],
                    )
                # scan: S[s] = (S[s-1] + data0[s]) * q[s]
                for h, (w0, wh) in enumerate(widths):
                    nc.vector.tensor_tensor_scan(
                        out=sw[:, w0 : w0 + wh],
                        data0=raw_pss[h][:, :wh],
                        data1=q[:, w0 : w0 + wh],
                        initial=0.0 if h == 0 else sw[:, w0 - 1 : w0],
                        op0=ADD,
                        op1=MULT,
                    )
                for h, (w0, wh) in enumerate(widths):
                    nc.vector.tensor_tensor_scan(
                        out=sy[:, w0 : w0 + wh],
                        data0=rawy_pss[h][:, :wh],
                        data1=q[:, w0 : w0 + wh],
                        initial=0.0 if h == 0 else sy[:, w0 - 1 : w0],
                        op0=ADD,
                        op1=MULT,
                    )
                nc.scalar.copy(out=wsum[:, tb : tb + 1], in_=sw[:, W - 1 : W])
                nc.scalar.copy(out=ynum[:, tb : tb + 1], in_=sy[:, W - 1 : W])

            # ---- finalize: yhat = clip(ynum / (wsum + 2e-6), .01, .99) ----
            wse = psmall.tile([P, NJ], F32, tag="wse")
            nc.vector.tensor_scalar(
                out=wse[:], in0=wsum[:], scalar1=2e-6, scalar2=None, op0=ADD
            )
            rcp = psmall.tile([P, NJ], F32, tag="rcp")
            nc.vector.reciprocal(out=rcp[:], in_=wse[:])
            yh = psmall.tile([P, NJ], F32, tag="yh")
            nc.vector.tensor_tensor(out=yh[:], in0=ynum[:], in1=rcp[:], op=MULT)
            yc = psmall.tile([P, NJ], F32, tag="yc")
            nc.vector.tensor_scalar(
                out=yc[:], in0=yh[:], scalar1=0.01, scalar2=0.99, op0=MAX, op1=MIN
            )
            nc.sync.dma_start(out=out[b], in_=yc[:])


def shard_inputs(y, problem_seq, embd_weight, lam_w, lam_b):
    """Build per-core input maps."""
    B = y.shape[0]
    assert B == N_CORES * NB_PER_CORE
    seq = np.ascontiguousarray(problem_seq).astype(np.int32)
    yf = np.ascontiguousarray(y).astype(np.float32)
    table = np.ascontiguousarray(embd_weight).astype(np.float32)
    lamw = np.ascontiguousarray(lam_w).reshape(P, 1).astype(np.float32)
    lamb = np.full((P, 1), np.float32(np.asarray(lam_b).reshape(-1)[0]))
    diag = np.eye(P, dtype=np.float32)
    smask = np.tril(np.ones((P, P), dtype=np.float32), k=-1)
    colv, rowv = np.meshgrid(np.arange(P), np.arange(P))
    minfw = np.where(colv < rowv, 1e30, -1.0).astype(np.float32)
    minfy = np.where(colv < rowv, 1e30, 0.0).astype(np.float32)

    in_maps = []
    for c in range(N_CORES):
        sl = slice(c * NB_PER_CORE, (c + 1) * NB_PER_CORE)
        # idx[b, p, j] = seq[b, j*128 + p]; ycol likewise
        idx = seq[sl].reshape(NB_PER_CORE, NJ, P).transpose(0, 2, 1)
        ycl = yf[sl].reshape(NB_PER_CORE, NJ, P).transpose(0, 2, 1)
        ybc_c = np.broadcast_to(yf[sl][:, None, :], (NB_PER_CORE, P, T))
        in_maps.append(
            {
                "table": table,
                "idx": np.ascontiguousarray(idx),
                "ybc": np.ascontiguousarray(ybc_c),
                "ycol": np.ascontiguousarray(ycl),
                "lamw": lamw,
                "lamb": lamb,
                "diag": diag,
                "smask": smask,
                "minfw": minfw,
                "minfy": minfy,
            }
        )
    return in_maps


def unshard_output(results):
    """results: list of 8 dicts with 'out' [4, 128, 8] -> yhat [32, 1024]."""
    parts = []
    for c in range(N_CORES):
        o = results[c]["out"]  # [NB, P, NJ]; yhat[b, j*128+p] = o[b, p, j]
        parts.append(o.transpose(0, 2, 1).reshape(NB_PER_CORE, T))
    return np.concatenate(parts, axis=0).astype(np.float32)


_NC_CACHE = None


def _get_program():
    global _NC_CACHE
    if _NC_CACHE is None:
        _NC_CACHE = build_program()
    return _NC_CACHE


def kernel(y, problem_seq, embd_weight, lam_w, lam_b, _trace=False, **trace_kwargs):
    nc = _get_program()
    in_maps = shard_inputs(y, problem_seq, embd_weight, lam_w, lam_b)
    res = run_bass_kernel_spmd(
        nc, in_maps, core_ids=list(range(N_CORES)), trace=_trace, **trace_kwargs
    )
    outp = unshard_output(res.results)
    if _trace:
        return outp, res
    return outp


if __name__ == "__main__":
    rng = np.random.default_rng(0)
    y = rng.random((32, T), dtype=np.float32)
    seq = rng.integers(0, N_VOCAB, size=(32, T)).astype(np.int32)
    emb = rng.standard_normal((N_VOCAB, P), dtype=np.float32)
    lw = (rng.standard_normal((P, 1), dtype=np.float32) / np.sqrt(P)).astype(np.float32)
    lb = (rng.standard_normal((1,), dtype=np.float32) * 0.01).astype(np.float32)
    outp = kernel(y, seq, emb, lw, lb)
    print("out", outp.shape, outp.dtype, outp[:2, :5])
